# revision 5
# baseline (speedup 1.0000x reference)
"""Trainium2 (8 NeuronCores, SPMD) kernel for windowed multi-head attention
with relative position bias (Swin-3D style block).

Strategy: pure data-parallel over the B=32 window axis — 4 windows per core,
no collectives. Per core:
  phase 1: qkv projection.  q,k produced TRANSPOSED (feature-on-partition)
           for the score matmuls; v produced natural (token-on-partition)
           with a column of ones appended (row 64 of the PV output then
           holds the softmax denominator for free).
  phase 2: per (head, window): scores^T = k_h @ q_h^T  (keys on partitions,
           queries on free axis), exp on ScalarE, multiply by
           exp(bias)^T (host-precomputed, resident in SBUF), PV matmul
           accumulating attn_out^T [64+1, 512], then normalize by the
           reciprocal of the sum row (broadcast via a K=1 outer-product
           matmul).
  phase 3: output projection from attn_out^T tiles; result lands natural
           [token, feature] and is DMAed out.

All matmul operands are bf16 (full TensorE rate); accumulation fp32 in PSUM.
The softmax scale is folded into the q weights on the host. exp(s+b) is
computed as exp(s)*exp(b) — exact up to fp rounding, and lets the bias add
run as a cheap bf16 4x-mode multiply on VectorE instead of an fp32 PSUM add.
v/proj biases enter the output linearly and are applied on the host (they
are zeros for this problem's inputs anyway).
"""

import numpy as np
import ml_dtypes

B, NTOK, DIM = 32, 512, 768
NH, HD = 12, 64
NCORES = 8
BW = B // NCORES          # 4 windows per core
SCALE = HD ** -0.5
KT = NTOK // 128          # 4 token tiles
FT = DIM // 128           # 6 feature tiles

TRACE = False             # set by test.py to capture neuron-profile timing
LAST_RESULT = None        # BassKernelResults of the last run (for profiling)

_nc_cache = {}


def _build(has_bqk: bool):
    import concourse.mybir as mybir
    import concourse.tile as tile
    from concourse import bacc
    from contextlib import ExitStack

    dt = mybir.dt
    bf16, f32 = dt.bfloat16, dt.float32
    AF = mybir.ActivationFunctionType

    # Bacc (not plain Bass): its compile pass splits multi-semaphore waits
    # into EventSemaphore instructions — TRN2 allows only 1 wait per inst.
    nc = bacc.Bacc("TRN2", target_bir_lowering=False, debug=False)
    xT_d = nc.declare_dram_parameter("xT", [BW, DIM, NTOK], bf16, False)
    wq_d = nc.declare_dram_parameter("wqkvT", [DIM, 3 * DIM], bf16, False)
    wp_d = nc.declare_dram_parameter("wprojT", [DIM, DIM], bf16, False)
    eb_d = nc.declare_dram_parameter("expb", [128, NH, KT, NTOK], bf16, False)
    bq_d = nc.declare_dram_parameter("bqk", [128, 2 * FT], f32, False)
    out_d = nc.declare_dram_parameter("out", [BW, NTOK, DIM], f32, True)

    ctx = ExitStack()
    with ctx:
        tc = ctx.enter_context(tile.TileContext(nc))
        const = ctx.enter_context(tc.tile_pool(name="const", bufs=1))
        xpool = ctx.enter_context(tc.tile_pool(name="xT", bufs=2))
        ebpool = ctx.enter_context(tc.tile_pool(name="expb", bufs=2))
        empool = ctx.enter_context(tc.tile_pool(name="expm", bufs=3))
        rpool = ctx.enter_context(tc.tile_pool(name="recip", bufs=2))
        rbpool = ctx.enter_context(tc.tile_pool(name="rb", bufs=2))
        opool = ctx.enter_context(tc.tile_pool(name="osb", bufs=3))
        ps_mm = ctx.enter_context(tc.tile_pool(name="ps_mm", bufs=4, space="PSUM"))
        ps_pv = ctx.enter_context(tc.tile_pool(name="ps_pv", bufs=2, space="PSUM"))
        ps_bc = ctx.enter_context(tc.tile_pool(name="ps_bc", bufs=2, space="PSUM"))

        wq_sb = const.tile([128, FT, 3 * DIM], bf16)
        nc.sync.dma_start(out=wq_sb, in_=wq_d[:, :].rearrange("(k p) c -> p k c", p=128))
        wp_sb = const.tile([128, FT, DIM], bf16)
        nc.sync.dma_start(out=wp_sb, in_=wp_d[:, :].rearrange("(k p) c -> p k c", p=128))
        bqk_sb = const.tile([128, 2 * FT], f32)
        nc.sync.dma_start(out=bqk_sb, in_=bq_d[:, :])
        ones_sb = const.tile([1, 64], f32)
        nc.vector.memset(ones_sb, 1.0)

        qkT = const.tile([128, BW, 2 * FT, NTOK], bf16)   # q,k transposed, per window
        vsb = const.tile([128, BW, KT, NH, HD + 1], bf16) # v natural + ones column
        nc.vector.memset(vsb, 1.0)                        # ones column survives the v copies
        aoT = const.tile([128, BW, FT, NTOK], bf16)       # attn output, transposed

        # ---- phase 1: qkv projections -------------------------------------
        for w in range(BW):
            xw = xpool.tile([128, FT, NTOK], bf16)
            nc.sync.dma_start(out=xw, in_=xT_d[w, :, :].rearrange("(k p) t -> p k t", p=128))
            for m in range(2 * FT):   # q,k features, transposed output
                ps = ps_mm.tile([128, 512], f32)
                for k in range(FT):
                    nc.tensor.matmul(
                        ps,
                        wq_sb[:, k, m * 128:(m + 1) * 128],
                        xw[:, k, :],
                        start=(k == 0), stop=(k == FT - 1),
                    )
                if has_bqk:
                    nc.scalar.activation(
                        out=qkT[:, w, m, :], in_=ps, func=AF.Identity,
                        bias=bqk_sb[:, m:m + 1], scale=1.0,
                    )
                else:
                    nc.vector.tensor_copy(out=qkT[:, w, m, :], in_=ps)
            for mt in range(KT):      # v natural layout
                for n in range(2):
                    ps = ps_mm.tile([128, 512], f32)
                    for k in range(FT):
                        nc.tensor.matmul(
                            ps[:, 0:384],
                            xw[:, k, mt * 128:(mt + 1) * 128],
                            wq_sb[:, k, 2 * DIM + n * 384: 2 * DIM + (n + 1) * 384],
                            start=(k == 0), stop=(k == FT - 1),
                        )
                    nc.vector.tensor_copy(
                        out=vsb[:, w, mt, n * 6:(n + 1) * 6, 0:HD],
                        in_=ps[:, 0:384].rearrange("p (j c) -> p j c", c=HD),
                    )

        # ---- phase 2: attention, software-pipelined by one (h, w) step ----
        def emit_pv(st):
            h, w, expm, pv_ps = st
            po = (h % 2) * 64
            mq = h // 2
            for kt in range(KT):
                nc.tensor.matmul(
                    pv_ps[0:HD + 1, :],
                    vsb[:, w, kt, h, :],
                    expm[:, kt, :],
                    start=(kt == 0), stop=(kt == KT - 1),
                )
            rc = rpool.tile([1, 512], f32)
            nc.vector.reciprocal(out=rc, in_=pv_ps[HD:HD + 1, :])
            bc = ps_bc.tile([128, 512], f32)
            nc.tensor.matmul(bc[0:64, :], ones_sb[0:1, :], rc, start=True, stop=True)
            rb = rbpool.tile([64, 512], f32)
            nc.scalar.copy(out=rb, in_=bc[0:64, :])
            nc.vector.tensor_mul(
                out=aoT[po:po + 64, w, mq, :], in0=pv_ps[0:HD, :], in1=rb,
            )

        pending = None
        for h in range(NH):
            eb = ebpool.tile([128, KT, NTOK], bf16)
            nc.sync.dma_start(out=eb, in_=eb_d[:, h, :, :])
            po = (h % 2) * 64
            mq, mk = h // 2, FT + h // 2
            for w in range(BW):
                expm = empool.tile([128, KT, NTOK], bf16)
                for kt in range(KT):
                    ps = ps_mm.tile([128, 512], f32)
                    nc.tensor.matmul(
                        ps,
                        qkT[po:po + 64, w, mk, kt * 128:(kt + 1) * 128],
                        qkT[po:po + 64, w, mq, :],
                        start=True, stop=True,
                    )
                    nc.scalar.activation(out=expm[:, kt, :], in_=ps, func=AF.Exp)
                nc.vector.tensor_mul(out=expm, in0=expm, in1=eb)
                if pending is not None:
                    emit_pv(pending)
                pending = (h, w, expm, ps_pv.tile([128, 512], f32, name="pv", tag="pv"))
        emit_pv(pending)

        # ---- phase 3: output projection -----------------------------------
        for w in range(BW):
            for mt in range(KT):
                osb = opool.tile([128, DIM], f32)
                for n in range(2):
                    ps = ps_mm.tile([128, 512], f32)
                    for k in range(FT):
                        nc.tensor.matmul(
                            ps[:, 0:384],
                            aoT[:, w, k, mt * 128:(mt + 1) * 128],
                            wp_sb[:, k, n * 384:(n + 1) * 384],
                            start=(k == 0), stop=(k == FT - 1),
                        )
                    nc.vector.tensor_copy(out=osb[:, n * 384:(n + 1) * 384], in_=ps[:, 0:384])
                nc.sync.dma_start(out=out_d[w, mt * 128:(mt + 1) * 128, :], in_=osb)

    if not nc.is_finalized():
        nc.finalize()
    return nc


def _host_prep(x, Wqkv, bqkv, rel_pos_bias_table, rel_pos_index):
    bf16 = ml_dtypes.bfloat16
    x = np.asarray(x, np.float32)
    Wqkv = np.asarray(Wqkv, np.float32)
    bqkv = np.asarray(bqkv, np.float32)
    table = np.asarray(rel_pos_bias_table, np.float32)
    idx = np.asarray(rel_pos_index)

    wqkvT = Wqkv.T.copy()               # [768, 2304]
    wqkvT[:, :DIM] *= SCALE             # fold softmax scale into q weights
    wqkvT_bf = wqkvT.astype(bf16)

    bqk = bqkv[:2 * DIM].copy()
    bqk[:DIM] *= SCALE
    has_bqk = bool(np.any(bqk))
    bqk_packed = np.ascontiguousarray(bqk.reshape(2 * FT, 128).T, dtype=np.float32)

    # expb[p, h, kt, q] = exp(bias_h[q, k]) with k = kt*128+p  (scores are transposed)
    E = np.exp(table[idx])              # [q, k, h]
    eb = E.transpose(1, 2, 0)           # [k, h, q]
    eb = eb.reshape(KT, 128, NH, NTOK).transpose(1, 2, 0, 3)   # [p, h, kt, q]
    eb_bf = np.ascontiguousarray(eb, dtype=bf16)

    xT = x.reshape(NCORES, BW, NTOK, DIM).transpose(0, 1, 3, 2)  # [core, w, feat, tok]
    xT_bf = np.ascontiguousarray(xT, dtype=bf16)
    return xT_bf, wqkvT_bf, bqk_packed, has_bqk, eb_bf


def kernel(x, Wqkv, bqkv, rel_pos_bias_table, rel_pos_index, Wproj, bproj):
    global LAST_RESULT
    from concourse.bass_utils import run_bass_kernel_spmd

    Wproj = np.asarray(Wproj, np.float32)
    bproj = np.asarray(bproj, np.float32)
    bqkv_np = np.asarray(bqkv, np.float32)

    xT_bf, wqkvT_bf, bqk_packed, has_bqk, eb_bf = _host_prep(
        x, Wqkv, bqkv_np, rel_pos_bias_table, rel_pos_index
    )
    wprojT_bf = np.ascontiguousarray(Wproj.T, dtype=ml_dtypes.bfloat16)

    key = has_bqk
    if key not in _nc_cache:
        _nc_cache[key] = _build(has_bqk)
    nc = _nc_cache[key]

    in_maps = [
        {
            "xT": xT_bf[c],
            "wqkvT": wqkvT_bf,
            "wprojT": wprojT_bf,
            "expb": eb_bf,
            "bqk": bqk_packed,
        }
        for c in range(NCORES)
    ]
    res = run_bass_kernel_spmd(
        nc, in_maps, list(range(NCORES)),
        trace=TRACE, trace_cores=[0] if TRACE else None,
    )
    LAST_RESULT = res
    out = np.concatenate([res.results[c]["out"] for c in range(NCORES)], axis=0)

    # v-bias and proj-bias enter the output linearly; apply exactly on host.
    corr = bproj + bqkv_np[2 * DIM:] @ Wproj.T
    if np.any(corr):
        out = out + corr.astype(np.float32)
    return np.ascontiguousarray(out, dtype=np.float32)


# revision 17
# speedup vs baseline: 1.2171x; 1.2171x over previous
"""Trainium2 (8 NeuronCores, SPMD) kernel for windowed multi-head attention
with relative position bias (Swin-3D style block).

Strategy: pure data-parallel over the B=32 window axis — 4 windows per core,
no collectives. Per core:
  phase 1: qkv projection.  q,k produced TRANSPOSED (feature-on-partition)
           for the score matmuls; v produced natural (token-on-partition)
           with a column of ones appended (row 64 of the PV output then
           holds the softmax denominator for free).
  phase 2: per (head, window): scores^T = k_h @ q_h^T  (keys on partitions,
           queries on free axis), exp on ScalarE, multiply by
           exp(bias)^T (host-precomputed, resident in SBUF), PV matmul
           accumulating attn_out^T [64+1, 512], then normalize by the
           reciprocal of the sum row (broadcast via a K=1 outer-product
           matmul).
  phase 3: output projection from attn_out^T tiles; result lands natural
           [token, feature] and is DMAed out.

All matmul operands are bf16 (full TensorE rate); accumulation fp32 in PSUM.
The softmax scale is folded into the q weights on the host. exp(s+b) is
computed as exp(s)*exp(b) — exact up to fp rounding, and lets the bias add
run as a cheap bf16 4x-mode multiply on VectorE instead of an fp32 PSUM add.
v/proj biases enter the output linearly and are applied on the host (they
are zeros for this problem's inputs anyway).
"""

import numpy as np
import ml_dtypes

B, NTOK, DIM = 32, 512, 768
NH, HD = 12, 64
NCORES = 8
BW = B // NCORES          # 4 windows per core
SCALE = HD ** -0.5
KT = NTOK // 128          # 4 token tiles
FT = DIM // 128           # 6 feature tiles

TRACE = False             # set by test.py to capture neuron-profile timing
LAST_RESULT = None        # BassKernelResults of the last run (for profiling)

_nc_cache = {}


def _build(has_bqk: bool):
    import concourse.mybir as mybir
    import concourse.tile as tile
    from concourse import bacc
    from contextlib import ExitStack

    dt = mybir.dt
    bf16, f32 = dt.bfloat16, dt.float32
    AF = mybir.ActivationFunctionType

    # Bacc (not plain Bass): its compile pass splits multi-semaphore waits
    # into EventSemaphore instructions — TRN2 allows only 1 wait per inst.
    nc = bacc.Bacc("TRN2", target_bir_lowering=False, debug=False)
    xT_d = nc.declare_dram_parameter("xT", [BW, DIM, NTOK], bf16, False)
    wq_d = nc.declare_dram_parameter("wqkvT", [DIM, 3 * DIM], bf16, False)
    wp_d = nc.declare_dram_parameter("wprojT", [DIM, DIM], bf16, False)
    eb_d = nc.declare_dram_parameter("expb", [128, NH, KT, NTOK], bf16, False)
    bq_d = nc.declare_dram_parameter("bqk", [128, 2 * FT], f32, False)
    out_d = nc.declare_dram_parameter("out", [BW, NTOK, DIM], f32, True)

    ctx = ExitStack()
    with ctx:
        tc = ctx.enter_context(tile.TileContext(nc))
        const = ctx.enter_context(tc.tile_pool(name="const", bufs=1))
        xpool = ctx.enter_context(tc.tile_pool(name="xT", bufs=2))
        ebpool = ctx.enter_context(tc.tile_pool(name="expb", bufs=2))
        empool = ctx.enter_context(tc.tile_pool(name="expm", bufs=4))
        rpool = ctx.enter_context(tc.tile_pool(name="recip", bufs=2))
        rbpool = ctx.enter_context(tc.tile_pool(name="rb", bufs=2))
        opool = ctx.enter_context(tc.tile_pool(name="osb", bufs=2))
        ps_mm = ctx.enter_context(tc.tile_pool(name="ps_mm", bufs=4, space="PSUM"))
        ps_pv = ctx.enter_context(tc.tile_pool(name="ps_pv", bufs=2, space="PSUM"))
        ps_bc = ctx.enter_context(tc.tile_pool(name="ps_bc", bufs=2, space="PSUM"))

        wq_sb = const.tile([128, FT, 3 * DIM], bf16)
        nc.sync.dma_start(out=wq_sb, in_=wq_d[:, :].rearrange("(k p) c -> p k c", p=128))
        wp_sb = const.tile([128, FT, DIM], bf16)
        nc.sync.dma_start(out=wp_sb, in_=wp_d[:, :].rearrange("(k p) c -> p k c", p=128))
        bqk_sb = const.tile([128, 2 * FT], f32)
        nc.sync.dma_start(out=bqk_sb, in_=bq_d[:, :])
        ones_sb = const.tile([1, 64], f32)
        nc.vector.memset(ones_sb, 1.0)

        qkT = const.tile([128, BW, 2 * FT, NTOK], bf16)   # q,k transposed, per window
        vsb = const.tile([128, BW, KT, NH, HD + 1], bf16) # v natural + ones column
        nc.vector.memset(vsb, 1.0)                        # ones column survives the v copies
        aoT = const.tile([128, BW, FT, NTOK], bf16)       # attn output, transposed

        # ---- phase 1: qkv projections -------------------------------------
        for w in range(BW):
            xw = xpool.tile([128, FT, NTOK], bf16)
            nc.sync.dma_start(out=xw, in_=xT_d[w, :, :].rearrange("(k p) t -> p k t", p=128))
            for m in range(2 * FT):   # q,k features, transposed output
                ps = ps_mm.tile([128, 512], f32)
                for k in range(FT):
                    nc.tensor.matmul(
                        ps,
                        wq_sb[:, k, m * 128:(m + 1) * 128],
                        xw[:, k, :],
                        start=(k == 0), stop=(k == FT - 1),
                    )
                if has_bqk:
                    nc.scalar.activation(
                        out=qkT[:, w, m, :], in_=ps, func=AF.Identity,
                        bias=bqk_sb[:, m:m + 1], scale=1.0,
                    )
                else:
                    nc.vector.tensor_copy(out=qkT[:, w, m, :], in_=ps)
            for mt in range(KT):      # v natural layout
                for n in range(2):
                    ps = ps_mm.tile([128, 512], f32)
                    for k in range(FT):
                        nc.tensor.matmul(
                            ps[:, 0:384],
                            xw[:, k, mt * 128:(mt + 1) * 128],
                            wq_sb[:, k, 2 * DIM + n * 384: 2 * DIM + (n + 1) * 384],
                            start=(k == 0), stop=(k == FT - 1),
                        )
                    nc.vector.tensor_copy(
                        out=vsb[:, w, mt, n * 6:(n + 1) * 6, 0:HD],
                        in_=ps[:, 0:384].rearrange("p (j c) -> p j c", c=HD),
                    )

        # ---- phase 2: attention ------------------------------------------
        # Heads are processed in even/odd pairs: the even head's q/k live on
        # partitions 0:64, the odd head's on 64:128, so interleaved score
        # matmuls hit disjoint PE row groups and run concurrently.  PV (and
        # the normalization) for a pair is emitted one pair late so the PE
        # always has score work while ACT/GpSimd run exp / bias-multiply.
        def emit_pv(st):
            h, w, expm = st
            po = (h % 2) * 64
            mq = h // 2
            pv_ps = ps_pv.tile([128, 512], f32, name="pv", tag="pv")
            for kt in range(KT):
                nc.tensor.matmul(
                    pv_ps[0:HD + 1, :],
                    vsb[:, w, kt, h, :],
                    expm[:, kt, :],
                    start=(kt == 0), stop=(kt == KT - 1),
                )
            ssb = rpool.tile([1, 512], f32, name="ssb", tag="ssb")
            nc.vector.tensor_copy(out=ssb, in_=pv_ps[HD:HD + 1, :])
            rc = rpool.tile([1, 512], f32)
            # reciprocal_approx_fast misreads PSUM sources — feed it from SBUF
            nc.vector.reciprocal_approx_fast(out=rc, in_=ssb)
            bc = ps_bc.tile([128, 512], f32, name="bc", tag="bc")
            nc.tensor.matmul(bc[0:64, :], ones_sb[0:1, :], rc, start=True, stop=True)
            rb = rbpool.tile([64, 512], f32)
            nc.scalar.copy(out=rb, in_=bc[0:64, :])
            nc.vector.tensor_mul(
                out=aoT[po:po + 64, w, mq, :], in0=pv_ps[0:HD, :], in1=rb,
            )

        pending = []
        for hp in range(NH // 2):
            eb = ebpool.tile([128, 2, KT, NTOK], bf16)
            nc.sync.dma_start(out=eb, in_=eb_d[:, 2 * hp:2 * hp + 2, :, :])
            for w in range(BW):
                em_e = empool.tile([128, KT, NTOK], bf16, name="em", tag="em")
                em_o = empool.tile([128, KT, NTOK], bf16, name="em", tag="em")
                for kt in range(KT):
                    for po, em in ((0, em_e), (64, em_o)):
                        ps = ps_mm.tile([128, 512], f32)
                        nc.tensor.matmul(
                            ps,
                            qkT[po:po + 64, w, FT + hp, kt * 128:(kt + 1) * 128],
                            qkT[po:po + 64, w, hp, :],
                            start=True, stop=True,
                        )
                        nc.scalar.activation(out=em[:, kt, :], in_=ps, func=AF.Exp)
                nc.vector.tensor_mul(out=em_e, in0=em_e, in1=eb[:, 0, :, :])
                nc.vector.tensor_mul(out=em_o, in0=em_o, in1=eb[:, 1, :, :])
                new = [(2 * hp, w, em_e), (2 * hp + 1, w, em_o)]
                for st in pending:
                    emit_pv(st)
                pending = new
        for st in pending:
            emit_pv(st)

        # ---- phase 3: output projection -----------------------------------
        for w in range(BW):
            for mt in range(KT):
                osb = opool.tile([128, DIM], f32)
                for n in range(2):
                    ps = ps_mm.tile([128, 512], f32)
                    for k in range(FT):
                        nc.tensor.matmul(
                            ps[:, 0:384],
                            aoT[:, w, k, mt * 128:(mt + 1) * 128],
                            wp_sb[:, k, n * 384:(n + 1) * 384],
                            start=(k == 0), stop=(k == FT - 1),
                        )
                    nc.vector.tensor_copy(out=osb[:, n * 384:(n + 1) * 384], in_=ps[:, 0:384])
                nc.sync.dma_start(out=out_d[w, mt * 128:(mt + 1) * 128, :], in_=osb)

    if not nc.is_finalized():
        nc.finalize()
    return nc


def _host_prep(x, Wqkv, bqkv, rel_pos_bias_table, rel_pos_index):
    bf16 = ml_dtypes.bfloat16
    x = np.asarray(x, np.float32)
    Wqkv = np.asarray(Wqkv, np.float32)
    bqkv = np.asarray(bqkv, np.float32)
    table = np.asarray(rel_pos_bias_table, np.float32)
    idx = np.asarray(rel_pos_index)

    wqkvT = Wqkv.T.copy()               # [768, 2304]
    wqkvT[:, :DIM] *= SCALE             # fold softmax scale into q weights
    wqkvT_bf = wqkvT.astype(bf16)

    bqk = bqkv[:2 * DIM].copy()
    bqk[:DIM] *= SCALE
    has_bqk = bool(np.any(bqk))
    bqk_packed = np.ascontiguousarray(bqk.reshape(2 * FT, 128).T, dtype=np.float32)

    # expb[p, h, kt, q] = exp(bias_h[q, k]) with k = kt*128+p  (scores are transposed)
    E = np.exp(table[idx])              # [q, k, h]
    eb = E.transpose(1, 2, 0)           # [k, h, q]
    eb = eb.reshape(KT, 128, NH, NTOK).transpose(1, 2, 0, 3)   # [p, h, kt, q]
    eb_bf = np.ascontiguousarray(eb, dtype=bf16)

    xT = x.reshape(NCORES, BW, NTOK, DIM).transpose(0, 1, 3, 2)  # [core, w, feat, tok]
    xT_bf = np.ascontiguousarray(xT, dtype=bf16)
    return xT_bf, wqkvT_bf, bqk_packed, has_bqk, eb_bf


def kernel(x, Wqkv, bqkv, rel_pos_bias_table, rel_pos_index, Wproj, bproj):
    global LAST_RESULT
    from concourse.bass_utils import run_bass_kernel_spmd

    Wproj = np.asarray(Wproj, np.float32)
    bproj = np.asarray(bproj, np.float32)
    bqkv_np = np.asarray(bqkv, np.float32)

    xT_bf, wqkvT_bf, bqk_packed, has_bqk, eb_bf = _host_prep(
        x, Wqkv, bqkv_np, rel_pos_bias_table, rel_pos_index
    )
    wprojT_bf = np.ascontiguousarray(Wproj.T, dtype=ml_dtypes.bfloat16)

    key = has_bqk
    if key not in _nc_cache:
        _nc_cache[key] = _build(has_bqk)
    nc = _nc_cache[key]

    in_maps = [
        {
            "xT": xT_bf[c],
            "wqkvT": wqkvT_bf,
            "wprojT": wprojT_bf,
            "expb": eb_bf,
            "bqk": bqk_packed,
        }
        for c in range(NCORES)
    ]
    res = run_bass_kernel_spmd(
        nc, in_maps, list(range(NCORES)),
        trace=TRACE, trace_cores=[0] if TRACE else None,
    )
    LAST_RESULT = res
    out = np.concatenate([res.results[c]["out"] for c in range(NCORES)], axis=0)

    # v-bias and proj-bias enter the output linearly; apply exactly on host.
    corr = bproj + bqkv_np[2 * DIM:] @ Wproj.T
    if np.any(corr):
        out = out + corr.astype(np.float32)
    return np.ascontiguousarray(out, dtype=np.float32)
